# revision 60
# baseline (speedup 1.0000x reference)
"""Trainium2 Bass kernel for nn_CausalLTXAttention (sliding-window + sink causal attention).

Sharding: 8 cores = 2 batches x 4 head-groups (4 heads / 256 inner cols each).
Each core computes column-parallel Q/K/V projections for its 256 inner cols over
the FULL sequence (no halo duplication), the rmsnorm sum-of-squares is completed
with a tiny (16KB) AllGather across the 4 cores of each batch, attention runs
banded (window 512 + sink) per head over all 2048 queries, and the output
projection is row-parallel over the core's 256 e-rows.  Partial outputs (bf16)
are summed on the host (plus bo).

Device layout notes:
  - raw q/k are projected in [l,e] layout; interleaved rope (norm weights /
    logit scale / 1/sqrt(dh) folded into host-precomputed cos/sin tables) is
    applied BEFORE the rmsnorm scale (commutes: rope is linear, scale is
    per-row); the scale arrives after the AllGather and is fused with the
    PE-transpose into qT/kT [e,l] tiles.
  - scores are computed transposed: S^T[k,q] in (k-tile x 256-query-chunk)
    pairs; the band structure means only ~6 k-tiles per chunk, with 6 static
    mask tiles (position-independent band offsets).  Scores run in f32r
    (fast PE mode, 256 moving dim); es/V/attn-out are bf16.  Softmax
    denominator is obtained by augmenting V with a ones column in the PV
    matmul; the reciprocal row is partition-broadcast on GpSimd.  Sink key 0
    lives in a dedicated 17th k-tile (column L replicates key 0, rest
    zeroed+masked) so it flows through the regular pair machinery.
  - engines are load-balanced: exp on ACT in 2-bank batches, rope/masks/
    copies spread across DVE/ACT/Pool, score/PV/projection matmuls + in-PSUM
    transposes on PE, one 16KB AllGather on the collective cores overlapped
    with the V projection.
"""

from contextlib import ExitStack

import numpy as np
import ml_dtypes

import concourse.bass as bass
import concourse.bacc as bacc
import concourse.mybir as mybir
import concourse.tile as tile
from concourse.bass_utils import run_bass_kernel_spmd
from concourse.masks import make_identity


# ---- problem constants (hardcoded per the harness contract) ----
B, L, D = 2, 2048, 2048
H, DH = 16, 64
INNER = H * DH  # 1024
WINDOW, SINK = 512, 1
EPS = 1e-6
NCORES = 8
NG = 4  # head groups (cores per batch)
EG = INNER // NG  # 256 inner cols per core
HG = H // NG  # 4 heads per core
NLT = L // 128  # 16 l-tiles
ND = D // 128  # 16 contraction d-tiles
NET = EG // 128  # 2 e-tiles per core
CH = 128  # query chunk for attention (128 minimizes band-tile waste: 5/query)
NCH = L // CH  # 16
VW = HG * (DH + 1)  # 260: v tiles with a ones column per head
NDC = D // 512  # 4 output d-chunks

F32 = mybir.dt.float32
F32R = mybir.dt.float32r
BF16 = mybir.dt.bfloat16
F8 = mybir.dt.float8e4
DR = mybir.MatmulPerfMode.DoubleRow
WSCALE = 64.0  # weights are quantized as fp8(64*W); 1/64 folded downstream

REPLICA_GROUPS = [[0, 1, 2, 3], [4, 5, 6, 7]]


def chunk_meta(c):
    """k-tiles covering the causal+window band for query chunk c, ordered so
    every masked tile sits at the tail, plus the combined-mask slice.

    Returns (tiles, tail, moff, mw): the trailing `tail` tiles are multiplied
    by mkc[:, moff:moff+mw] in one DVE op.  Tail orders: c>=5 -> [c-4 (j<p),
    c (j>=p), sink NLT (p==0)]; c==4 -> [0 ((j<p)|(p==0)), 4 (j>=p)];
    c<=3 -> [c (j>=p)].  (j = query offset in chunk, p = key offset.)
    """
    mid = list(range(max(0, c - 3), c))
    if c >= 5:
        return mid + [c - 4, c, NLT], 3, 0, 384
    if c == 4:
        return mid + [0, 4], 2, 384, 256
    return mid + [c], 1, 640, 128


def _build(with_bias: bool):
    nc = bacc.Bacc("TRN2", target_bir_lowering=False, debug=False, num_devices=NCORES)

    # hi/lo fp8 pairs; weights are packed [h | l] along the column axis so a
    # row's DMA run is 512B (the <512B run penalty doubles DMA cost)
    xh = nc.dram_tensor("xh", [D, L], F8, kind="ExternalInput")
    xl = nc.dram_tensor("xl", [D, L], F8, kind="ExternalInput")
    wq2 = nc.dram_tensor("wq2", [D, 2 * EG], F8, kind="ExternalInput")
    wk2 = nc.dram_tensor("wk2", [D, 2 * EG], F8, kind="ExternalInput")
    wv2 = nc.dram_tensor("wv2", [D, 2 * EG], F8, kind="ExternalInput")
    woT = nc.dram_tensor("woT", [EG, D], BF16, kind="ExternalInput")
    tabs_d = nc.dram_tensor("tabs", [L, 4, EG], BF16, kind="ExternalInput")
    mskd = nc.dram_tensor("msk", [128, 768], BF16, kind="ExternalInput")
    if with_bias:
        bqr = nc.dram_tensor("bqr", [1, EG], BF16, kind="ExternalInput")
        bkr = nc.dram_tensor("bkr", [1, EG], BF16, kind="ExternalInput")
        bvr = nc.dram_tensor("bvr", [1, EG], BF16, kind="ExternalInput")
    outp = nc.dram_tensor("outp", [L, D], BF16, kind="ExternalOutput")
    ssel = nc.dram_tensor("ssel", [128 * 32], F32, kind="Internal")
    ssag = nc.dram_tensor("ssag", [NG, 128 * 32], F32, kind="Internal")

    # partition-major views for blocked DMA loads
    xhv = xh.ap().rearrange("(t p) l -> p t l", p=128)  # [128, 16, 2048]
    xlv = xl.ap().rearrange("(t p) l -> p t l", p=128)
    wqv = wq2.ap().rearrange("(t p) e -> p t e", p=128)  # [128, 16, 512]
    wkv = wk2.ap().rearrange("(t p) e -> p t e", p=128)
    wvv = wv2.ap().rearrange("(t p) e -> p t e", p=128)
    wov = woT.ap().rearrange("(t p) d -> p t d", p=128)  # [128, 2, 2048]
    tbv = tabs_d.ap().rearrange("(lt p) f e -> p lt f e", p=128)  # [128, 16, 4, 256]

    with tile.TileContext(nc) as tc, ExitStack() as ctx:
        consts = ctx.enter_context(tc.tile_pool(name="consts", bufs=1))
        big = ctx.enter_context(tc.tile_pool(name="big", bufs=1))

        ident = consts.tile([128, 128], BF16, tag="ident", name="ident")
        make_identity(nc, ident)
        eps_t = consts.tile([128, 1], F32, tag="eps", name="eps")
        nc.vector.memset(eps_t, EPS)
        one_sc = consts.tile([128, 1], F32, tag="one_sc", name="one_sc")
        nc.vector.memset(one_sc, 1.0)
        ones4 = consts.tile([128, HG], F32, tag="ones4", name="ones4")
        nc.vector.memset(ones4, 1.0)
        mkc = consts.tile([128, 768], BF16, tag="mkc", name="mkc")
        if with_bias:
            ones_row = consts.tile([1, 128], BF16, tag="ones_row", name="ones_row")
            nc.vector.memset(ones_row, 1.0)
            b_rows = {}
            for nm, dram in (("q", bqr), ("k", bkr), ("v", bvr)):
                b_rows[nm] = consts.tile([1, EG], BF16, tag=f"b_{nm}", name=f"b_{nm}")
                nc.sync.dma_start(out=b_rows[nm], in_=dram.ap())

        # persistent tiles
        rq = [big.tile([128, EG], F32, tag=f"rq{i}", name=f"rq{i}") for i in range(NLT)]
        rk = [big.tile([128, EG], F32, tag=f"rk{i}", name=f"rk{i}") for i in range(NLT)]
        # vA[NLT] is the sink tile: zero except row 0 = copy of key 0's row
        vA = [
            big.tile([128, VW], BF16, tag=f"vA{i}", name=f"vA{i}")
            for i in range(NLT + 1)
        ]
        # combined q/k transposed tile, bf16: index j on the middle axis is
        # (0: k-et0, 1: k-et1, 2: q-et0, 3: q-et1).  k rows have an extra
        # 128-col sink tile at [L, L+128): col L = copy of key 0, rest zero
        # (masked out; zeros keep exp() of the dead region finite).
        qkT = big.tile([128, 4, L + 128], BF16, tag="qkT", name="qkT")
        aT = [big.tile([128, L], BF16, tag=f"aT{i}", name=f"aT{i}") for i in range(NET)]
        nc.vector.memset(vA[NLT].bitcast(F32), 0.0)
        for j in range(NET):
            nc.vector.memset(qkT[:, j, L : L + 128].bitcast(F32), 0.0)
        ss2 = big.tile([128, 2 * NLT], F32, tag="ss2", name="ss2")
        ssg = big.tile([128, NG, 2 * NLT], F32, tag="ssg", name="ssg")
        sst = big.tile([128, 2 * NLT], F32, tag="sst", name="sst")
        rr = big.tile([128, 2 * NLT], F32, tag="rr", name="rr")
        wog = big.tile([128, NET, D], BF16, tag="wog", name="wog")

        # ---- projection-phase pools (released before attention) ----
        pctx = ctx.enter_context(ExitStack())
        wp = pctx.enter_context(tc.tile_pool(name="wp", bufs=1))
        xp = pctx.enter_context(tc.tile_pool(name="xp", bufs=1))
        tabsp = pctx.enter_context(tc.tile_pool(name="tabsp", bufs=1))
        work = pctx.enter_context(tc.tile_pool(name="work", bufs=1))
        psP = pctx.enter_context(tc.tile_pool(name="psP", bufs=1, space="PSUM"))

        # x stays resident in SBUF for both phases (64KB/partition fp8 hi+lo);
        # loaded in 512-l-col chunks so every DMA run is >=512B.
        xgh = xp.tile([128, ND, L], F8, tag="xgh", name="xgh")
        xgl = xp.tile([128, ND, L], F8, tag="xgl", name="xgl")

        def load_xchunk(ci):
            cs = slice(ci * 512, (ci + 1) * 512)
            nc.sync.dma_start(out=xgh[:, :, cs], in_=xhv[:, :, cs])
            nc.sync.dma_start(out=xgl[:, :, cs], in_=xlv[:, :, cs])

        def load_tb(lt):
            tb = tabsp.tile([128, 4, EG], BF16, tag="tb", bufs=6, name="tb")
            nc.sync.dma_start(out=tb, in_=tbv[:, lt])
            return tb

        # DMA issue order is SP-queue order: q weights + first x tiles first
        # (quartered, so the first contraction matmuls can start early),
        # the rest behind them.
        wq_g = wp.tile([128, ND, 2 * EG], F8, tag="wq", name="wq")
        wk_g = wp.tile([128, ND, 2 * EG], F8, tag="wk", name="wk")
        wv_g = wp.tile([128, ND, 2 * EG], F8, tag="wv", name="wv")
        nc.sync.dma_start(out=wq_g[:, 0:4], in_=wqv[:, 0:4])
        nc.sync.dma_start(out=xgh[:, 0:4, 0:512], in_=xhv[:, 0:4, 0:512])
        nc.sync.dma_start(out=xgl[:, 0:4, 0:512], in_=xlv[:, 0:4, 0:512])
        nc.sync.dma_start(out=wq_g[:, 4:16], in_=wqv[:, 4:16])
        nc.sync.dma_start(out=xgh[:, 4:16, 0:512], in_=xhv[:, 4:16, 0:512])
        nc.sync.dma_start(out=xgl[:, 4:16, 0:512], in_=xlv[:, 4:16, 0:512])
        nc.sync.dma_start(out=wk_g, in_=wkv)
        tbs_pre = [load_tb(lt) for lt in range(4)]

        def proj_psum(lt, wg, bias_key):
            # fp8 hi/lo: x ~= xh+xl, 64W ~= wh+wl; psum = 64*(x@W) via
            # xh*wh + xh*wl + xl*wh (xl*wl dropped, ~0.4% of a product).
            # DoubleRow packs 2 contraction d-tiles per matmul at 0.5 cyc/row.
            ps = psP.tile([128, EG], F32, tag="pp", bufs=8, name="pp")
            ssl = slice(lt * 128, (lt + 1) * 128)
            for p8 in range(ND // 2):
                d2 = slice(2 * p8, 2 * p8 + 2)
                nc.tensor.matmul(
                    ps, lhsT=xgh[:, d2, ssl], rhs=wg[:, d2, 0:EG],
                    start=(p8 == 0), stop=False, perf_mode=DR,
                )
                nc.tensor.matmul(
                    ps, lhsT=xgh[:, d2, ssl], rhs=wg[:, d2, EG : 2 * EG],
                    start=False, stop=False, perf_mode=DR,
                )
                nc.tensor.matmul(
                    ps, lhsT=xgl[:, d2, ssl], rhs=wg[:, d2, 0:EG],
                    start=False, stop=(p8 == ND // 2 - 1 and not with_bias),
                    perf_mode=DR,
                )
            if with_bias:
                nc.tensor.matmul(
                    ps, lhsT=ones_row, rhs=b_rows[bias_key], start=False, stop=True
                )
            return ps

        # ---------------- phase A: raw q/k projections + rope + partial SS ----
        # squares + rope read the projection PSUM directly (no SBUF staging
        # copy): ACT does the square-accumulate, DVE the rope multiplies.
        for ci in range(4):
            tbs = tbs_pre if ci == 0 else tbs_next
            if ci < 3:
                load_xchunk(ci + 1)
                tbs_next = [load_tb(4 * (ci + 1) + j) for j in range(4)]
            if ci == 2:  # v weights not needed until phase B
                nc.sync.dma_start(out=wv_g, in_=wvv)
            # Q for both l-tiles of a pair BEFORE either K: at startup the
            # first K weights are still in flight behind wq/x, so this gives
            # the PE some Q work to cover the wk transfer.
            for half in range(2):
                lts = (4 * ci + 2 * half, 4 * ci + 2 * half + 1)
                psqs = [proj_psum(lt, wq_g, "q") for lt in lts]
                psks = [proj_psum(lt, wk_g, "k") for lt in lts]
                for sub in range(2):
                    lt = lts[sub]
                    tb = tbs[2 * half + sub]
                    for ps, dst, ti, ss_col in (
                        (psqs[sub], rq[lt], 0, lt),
                        (psks[sub], rk[lt], 2, NLT + lt),
                    ):
                        sq = work.tile([128, EG], F32, tag="sq", bufs=2, name="sq")
                        nc.scalar.activation(
                            sq, ps, mybir.ActivationFunctionType.Square,
                            accum_out=ss2[:, ss_col : ss_col + 1],
                        )
                        tch = tb[:, ti]
                        tsh = tb[:, ti + 1]
                        # rope split across DVE (strided swap muls) and Pool
                        # (straight mul+add); ACT holds the square-accums
                        tmp = work.tile([128, EG], F32, tag="ropetmp", bufs=2, name="ropetmp")
                        nc.vector.tensor_mul(tmp[:, 0::2], ps[:, 1::2], tsh[:, 0::2])
                        nc.vector.tensor_mul(tmp[:, 1::2], ps[:, 0::2], tsh[:, 1::2])
                        nc.vector.tensor_mul(dst, ps, tch)
                        nc.gpsimd.tensor_add(dst, dst, tmp)

        # ---- rmsnorm sum-of-squares completion across the 4-core group ----
        # staging transfers ride the fast HWDGE (sync) queue; phase B no
        # longer loads x, so there is no head-of-line risk there.  wog/masks
        # are queued between them so they aren't stuck behind the
        # collective-dependent gather-in.
        sselv = ssel.ap().rearrange("(p j) -> p j", p=128)  # [128, 32]
        nc.sync.dma_start(out=sselv, in_=ss2)
        nc.gpsimd.collective_compute(
            kind="AllGather",
            op=mybir.AluOpType.bypass,
            replica_groups=REPLICA_GROUPS,
            ins=[ssel.ap()],
            outs=[ssag.ap()],
        )
        nc.sync.dma_start(out=wog, in_=wov)
        nc.sync.dma_start(out=mkc, in_=mskd.ap())
        nc.sync.dma_start(out=ssg, in_=ssag.ap().rearrange("g (p j) -> p g j", p=128))

        # ---------------- phase B: v projection (overlaps the AllGather) ----
        # psum->vA copies on ACT (idle during this phase; on DVE they would
        # delay the rs chain).  The rs = 1/sqrt(mean(ss)+eps) block is
        # injected near the END of phase B: by then the AllGather result is
        # long since landed, so neither ACT nor DVE blocks on it, and rr is
        # ready just before the attention transposes need it.
        for lt in range(NLT):
            if lt == NLT - 4:
                nc.vector.tensor_add(sst, ssg[:, 0], ssg[:, 1])
                nc.vector.tensor_add(sst, sst, ssg[:, 2])
                nc.vector.tensor_add(sst, sst, ssg[:, 3])
                nc.scalar.activation(
                    rr, sst, mybir.ActivationFunctionType.Sqrt,
                    bias=eps_t, scale=1.0 / (INNER * WSCALE * WSCALE),
                )
                nc.vector.reciprocal(rr, rr)
                # dummy Exp: pulls the exp-table load (1.3us) into phase B's
                # ACT idle window instead of the first attention exp.  Both
                # the sqrt and exp tables serve Square/Copy, so these are the
                # only two switches and neither is on the critical path.
                warm = work.tile([1, 1], F32, tag="warm", bufs=1, name="warm")
                nc.scalar.activation(
                    warm, eps_t[0:1, :], mybir.ActivationFunctionType.Exp
                )
            psv = proj_psum(lt, wv_g, "v")
            vA_r = vA[lt].rearrange("p (h c) -> p h c", c=DH + 1)
            nc.scalar.activation(
                vA_r[:, :, 0:DH],
                psv.rearrange("p (h c) -> p h c", c=DH),
                mybir.ActivationFunctionType.Copy,
                scale=1.0 / WSCALE,
            )
            nc.vector.tensor_scalar_mul(vA_r[:, :, DH], ones4, one_sc)
            if lt == 0:
                nc.gpsimd.tensor_copy(vA[NLT][0:1, :], vA[0][0:1, :])

        # ---- release projection pools; open attention/output pools ----
        # PSUM banks are 2KB/partition and pool buffers are bank-granular, so
        # [128, 256]-shaped psums are packed two-per-bank: psS banks hold two
        # k-tiles' scores, psO banks hold two heads' PV outputs, and psP2
        # banks serve both the out-projection and (quartered) the transposes.
        pctx.close()
        esp = ctx.enter_context(tc.tile_pool(name="esp", bufs=1))
        awork = ctx.enter_context(tc.tile_pool(name="awork", bufs=1))
        outw = ctx.enter_context(tc.tile_pool(name="outw", bufs=1))
        # PSUM budget (8 banks): pss 5 (3 manual 768-col slots) + po 1 +
        # pso 1 + ptq(bf16) 1.  Manual sub-bank slots (pool buffers are
        # bank-granular); slot rotation keeps WAR windows wide.  3 score
        # slots let the PE run scores ~3 heads ahead so exps go
        # back-to-back on ACT - the exp pace IS the chunk pace.
        psA = ctx.enter_context(tc.tile_pool(name="psA", bufs=1, space="PSUM"))
        pss_all = psA.tile([128, 2560], F32, tag="pss", bufs=1, name="pss")
        pso_all = psA.tile([DH + 1, 512], F32, tag="pso", bufs=1, name="pso")
        ptq_all = psA.tile([128, 1024], BF16, tag="ptq", bufs=1, name="ptq")
        po_all = psA.tile([128, 512], F32, tag="po", bufs=1, name="po")

        def scale_transpose(lt):
            """kn/qn scale by rs (to bf16), 4 PE transposes through one psum
            half-bank, then a single strided copy into the combined qkT tile."""
            ptq = ptq_all[:, (lt % 2) * 512 : (lt % 2 + 1) * 512]
            for qi, (src_t, col) in enumerate(
                ((rk[lt], NLT + lt), (rq[lt], lt))
            ):
                n = awork.tile([128, EG], BF16, tag="qkn", bufs=4, name="qkn")
                nc.gpsimd.tensor_scalar_mul(n, src_t, rr[:, col : col + 1])
                for et in range(NET):
                    q4 = slice((2 * qi + et) * 128, (2 * qi + et + 1) * 128)
                    nc.tensor.transpose(
                        ptq[:, q4], n[:, et * 128 : (et + 1) * 128], ident
                    )
            nc.vector.tensor_copy(qkT[:, :, lt * 128 : (lt + 1) * 128], ptq)
            if lt == 0:
                for j in range(NET):
                    nc.gpsimd.tensor_copy(qkT[:, j, L : L + 1], qkT[:, j, 0:1])

        def outproj_piece(lt, dc):
            # single po slot: pieces are spread across the chunk as PE
            # filler, so the copy->next-matmul WAR serialization is hidden;
            # copies alternate DVE/ACT (the only PSUM readers)
            for et in range(NET):
                nc.tensor.matmul(
                    po_all,
                    lhsT=aT[et][:, lt * 128 : (lt + 1) * 128],
                    rhs=wog[:, et, dc * 512 : (dc + 1) * 512],
                    start=(et == 0),
                    stop=(et == NET - 1),
                )
            osb = outw.tile([128, 512], BF16, tag="osb", bufs=4, name="osb")
            if dc % 2 == 0:
                nc.vector.tensor_copy(osb, po_all)
            else:
                nc.scalar.copy(osb, po_all)
            nc.sync.dma_start(
                out=outp.ap()[lt * 128 : (lt + 1) * 128, dc * 512 : (dc + 1) * 512],
                in_=osb,
            )

        # ---------------- attention, pipelined by query chunk ----------------
        # scale+transposes for chunk c+1 are emitted during chunk c so the
        # DVE-scale -> PE-transpose -> DVE-copy chain hides behind attention.
        def scores(cx, h):
            c, (tiles, tail, moff, mw), cs, es_tiles, _ = cx
            et, r0 = h // 2, (h % 2) * 64
            # one psum tile + one exp instruction for the whole band (<=6
            # tiles); masked tiles sit at the tail so ONE combined-mask mul
            # covers them all
            n = len(tiles)
            slot = (4 * c + h) % 3
            pss = pss_all[:, slot * 768 : slot * 768 + 768]
            esb = esp.tile([128, 768], BF16, tag="es", bufs=6, name="es")
            for idx, t in enumerate(tiles):
                hs = slice(idx * CH, (idx + 1) * CH)
                nc.tensor.matmul(
                    pss[:, hs],
                    lhsT=qkT[r0 : r0 + 64, et, t * 128 : (t + 1) * 128],
                    rhs=qkT[r0 : r0 + 64, 2 + et, cs],
                    start=True,
                    stop=True,
                    skip_group_check=True,
                )
                es_tiles[(h, t)] = (esb, hs)
            nc.scalar.activation(
                esb[:, 0 : n * CH], pss[:, 0 : n * CH],
                mybir.ActivationFunctionType.Exp,
            )
            s0 = (n - tail) * CH
            nc.vector.tensor_mul(
                esb[:, s0 : s0 + mw], esb[:, s0 : s0 + mw], mkc[:, moff : moff + mw]
            )

        def pv(cx, h):
            c, (tiles, tail, moff, mw), cs, es_tiles, pso_banks = cx
            et, r0 = h // 2, (h % 2) * 64
            off = (h // 2) * 256 + (h % 2) * CH
            pso = pso_all[:, off : off + CH]
            for i, t in enumerate(tiles):
                esb, hs = es_tiles[(h, t)]
                nc.tensor.matmul(
                    pso,
                    lhsT=vA[t][:, h * (DH + 1) : (h + 1) * (DH + 1)],
                    rhs=esb[:, hs],
                    start=(i == 0),
                    stop=(i == len(tiles) - 1),
                    skip_group_check=True,
                )

        def denom(cx, half):
            # reciprocal over 2 heads' ones-column row, one Pool broadcast,
            # then per-head normalize muls on DVE (psum reads are ACT/DVE only)
            c, _, cs, es_tiles, pso_banks = cx
            pso = pso_all[:, half * 256 : half * 256 + 2 * CH]
            rec = awork.tile([1, 2 * CH], F32, tag="rec", bufs=3, name="rec")
            nc.vector.reciprocal(rec, pso[DH : DH + 1, :])
            rb = awork.tile([64, 2 * CH], F32, tag="rb", bufs=3, name="rb")
            nc.gpsimd.partition_broadcast(rb, rec)
            for hh in range(2):
                h = 2 * half + hh
                et, r0 = h // 2, (h % 2) * 64
                nc.vector.tensor_mul(
                    aT[et][r0 : r0 + 64, cs],
                    pso[0:DH, hh * CH : (hh + 1) * CH],
                    rb[:, hh * CH : (hh + 1) * CH],
                )

        def mkcx(c):
            return (c, chunk_meta(c), slice(c * CH, (c + 1) * CH), {}, {})

        # the lagged out-projection of chunk c-1 is interleaved into chunk c's
        # score/PV sequence: its matmuls depend only on chunk c-1's aT, so
        # they are ideal PE filler while PV waits on the exp/mask chain.
        scale_transpose(0)
        scale_transpose(1)
        for c in range(NCH):
            if c + 2 < NCH:
                scale_transpose(c + 2)
            cx = mkcx(c)
            # PE queue is in-order: interleave so nothing head-of-line blocks.
            # scores(h+3) waits on exp(h) freeing its pss slot; outproj
            # pieces are filler between score/pv groups.
            scores(cx, 0)
            scores(cx, 1)
            if c >= 1:
                outproj_piece(c - 1, 0)
            scores(cx, 2)
            pv(cx, 0)
            if c >= 1:
                outproj_piece(c - 1, 1)
            scores(cx, 3)
            pv(cx, 1)
            denom(cx, 0)
            if c >= 1:
                outproj_piece(c - 1, 2)
            pv(cx, 2)
            pv(cx, 3)
            denom(cx, 1)
            if c >= 1:
                outproj_piece(c - 1, 3)
        for dc in range(NDC):
            outproj_piece(NCH - 1, dc)

    nc.compile()
    return nc


_NC_CACHE = {}


def get_nc(with_bias: bool):
    if with_bias not in _NC_CACHE:
        _NC_CACHE[with_bias] = _build(with_bias)
    return _NC_CACHE[with_bias]


def _fold_tables(cosf, sinf, w):
    """Fold rmsnorm weight w (per channel) into interleaved-rope cos/sin tables.

    Kernel computes raw*tc + swap(raw)*ts with swap pairing (odd->even,
    even->odd), so:
      tc[:, e]    = cos[:, e]    * w[e]
      ts[:, 2i]   = -sin[:, 2i]  * w[2i+1]
      ts[:, 2i+1] =  sin[:, 2i+1]* w[2i]
    """
    tc_ = cosf * w[None, :]
    ts_ = np.empty_like(sinf)
    ts_[:, 0::2] = -sinf[:, 0::2] * w[None, 1::2]
    ts_[:, 1::2] = sinf[:, 1::2] * w[None, 0::2]
    return tc_, ts_


def _build_masks():
    """Concatenated mask strip [128, 768]: [j<p | j>=p | p==0] for c>=5 at 0,
    [(j<p)|(p==0) | j>=p] for c==4 at 384, [j>=p] for c<=3 at 640."""
    p = np.arange(128)[:, None]
    j = np.arange(CH)[None, :]
    lower = (j < p).astype(np.float32)
    diag = (j >= p).astype(np.float32)
    sink = np.broadcast_to((p == 0).astype(np.float32), (128, CH))
    lower_sink = ((j < p) | (p == 0)).astype(np.float32)
    strip = np.concatenate([lower, diag, sink, lower_sink, diag, diag], axis=1)
    return np.ascontiguousarray(strip).astype(ml_dtypes.bfloat16)


def make_in_maps(inputs):
    f = lambda k: np.asarray(inputs[k], np.float32)
    x = f("x")
    cosf = f("cos")[0]
    sinf = f("sin")[0]
    lls = f("logit_log_scale")[0, :, 0]
    bq, bk, bv = f("bq"), f("bk"), f("bv")
    with_bias = bool(np.any(bq) or np.any(bk) or np.any(bv))

    qtc_f, qts_f = _fold_tables(cosf, sinf, f("qn_w"))
    ktc_f, kts_f = _fold_tables(cosf, sinf, f("kn_w"))
    qs = (lls * (1.0 / np.sqrt(DH)))[:, None].astype(np.float32)
    # tables absorb the 1/WSCALE that undoes the fp8 weight scaling
    tabs_full = np.stack(
        [qtc_f * qs, qts_f * qs, ktc_f, kts_f], axis=1
    ) * (1.0 / WSCALE)  # [L, 4, INNER]

    Wq, Wk, Wv, Wo = f("Wq"), f("Wk"), f("Wv"), f("Wo")
    msk = _build_masks()

    FP8 = ml_dtypes.float8_e4m3

    def hilo(a):
        h = a.astype(FP8)
        l = (a - h.astype(np.float32)).astype(FP8)
        return h, l

    x_b = []
    for b in range(B):
        xh_, xl_ = hilo(np.ascontiguousarray(x[b].T))
        x_b.append({"xh": xh_, "xl": xl_})
    per_g = []
    for g in range(NG):
        sl = slice(g * EG, (g + 1) * EG)
        gm = {"msk": msk}
        for nm, W in (("wq", Wq), ("wk", Wk), ("wv", Wv)):
            h, l = hilo(np.ascontiguousarray(W[sl].T) * WSCALE)
            gm[nm + "2"] = np.ascontiguousarray(np.concatenate([h, l], axis=1))
        gm["woT"] = np.ascontiguousarray(Wo[:, sl].T).astype(ml_dtypes.bfloat16)
        gm["tabs"] = np.ascontiguousarray(tabs_full[:, :, sl]).astype(ml_dtypes.bfloat16)
        if with_bias:
            gm["bqr"] = (WSCALE * bq)[None, sl].astype(ml_dtypes.bfloat16)
            gm["bkr"] = (WSCALE * bk)[None, sl].astype(ml_dtypes.bfloat16)
            gm["bvr"] = (WSCALE * bv)[None, sl].astype(ml_dtypes.bfloat16)
        per_g.append(gm)

    ims = []
    for c in range(NCORES):
        b, g = divmod(c, NG)
        ims.append({**x_b[b], **per_g[g]})
    return ims, with_bias


last_results = None


def kernel(**inputs):
    global last_results
    ims, with_bias = make_in_maps(inputs)
    nc = get_nc(with_bias)
    res = run_bass_kernel_spmd(nc, ims, core_ids=list(range(NCORES)))
    last_results = res
    out = np.zeros((B, L, D), np.float32)
    for c, om in enumerate(res.results):
        out[c // NG] += np.asarray(om["outp"], dtype=np.float32)
    out += np.asarray(inputs["bo"], np.float32)[None, None, :]
    return out



# revision 62
# speedup vs baseline: 1.0098x; 1.0098x over previous
"""Trainium2 Bass kernel for nn_CausalLTXAttention (sliding-window + sink causal attention).

Sharding: 8 cores = 2 batches x 4 head-groups (4 heads / 256 inner cols each).
Each core computes column-parallel Q/K/V projections for its 256 inner cols over
the FULL sequence (no halo duplication), the rmsnorm sum-of-squares is completed
with a tiny (16KB) AllGather across the 4 cores of each batch, attention runs
banded (window 512 + sink) per head over all 2048 queries, and the output
projection is row-parallel over the core's 256 e-rows.  Partial outputs (bf16)
are summed on the host (plus bo).

Device layout notes:
  - raw q/k are projected in [l,e] layout; interleaved rope (norm weights /
    logit scale / 1/sqrt(dh) folded into host-precomputed cos/sin tables) is
    applied BEFORE the rmsnorm scale (commutes: rope is linear, scale is
    per-row); the scale arrives after the AllGather and is fused with the
    PE-transpose into qT/kT [e,l] tiles.
  - scores are computed transposed: S^T[k,q] in (k-tile x 256-query-chunk)
    pairs; the band structure means only ~6 k-tiles per chunk, with 6 static
    mask tiles (position-independent band offsets).  Scores run in f32r
    (fast PE mode, 256 moving dim); es/V/attn-out are bf16.  Softmax
    denominator is obtained by augmenting V with a ones column in the PV
    matmul; the reciprocal row is partition-broadcast on GpSimd.  Sink key 0
    lives in a dedicated 17th k-tile (column L replicates key 0, rest
    zeroed+masked) so it flows through the regular pair machinery.
  - engines are load-balanced: exp on ACT in 2-bank batches, rope/masks/
    copies spread across DVE/ACT/Pool, score/PV/projection matmuls + in-PSUM
    transposes on PE, one 16KB AllGather on the collective cores overlapped
    with the V projection.
"""

from contextlib import ExitStack

import numpy as np
import ml_dtypes

import concourse.bass as bass
import concourse.bacc as bacc
import concourse.mybir as mybir
import concourse.tile as tile
from concourse.bass_utils import run_bass_kernel_spmd
from concourse.masks import make_identity


# ---- problem constants (hardcoded per the harness contract) ----
B, L, D = 2, 2048, 2048
H, DH = 16, 64
INNER = H * DH  # 1024
WINDOW, SINK = 512, 1
EPS = 1e-6
NCORES = 8
NG = 4  # head groups (cores per batch)
EG = INNER // NG  # 256 inner cols per core
HG = H // NG  # 4 heads per core
NLT = L // 128  # 16 l-tiles
ND = D // 128  # 16 contraction d-tiles
NET = EG // 128  # 2 e-tiles per core
CH = 128  # query chunk for attention (128 minimizes band-tile waste: 5/query)
NCH = L // CH  # 16
VW = HG * (DH + 1)  # 260: v tiles with a ones column per head
NDC = D // 512  # 4 output d-chunks

F32 = mybir.dt.float32
F32R = mybir.dt.float32r
BF16 = mybir.dt.bfloat16
F8 = mybir.dt.float8e4
DR = mybir.MatmulPerfMode.DoubleRow
WSCALE = 64.0  # weights are quantized as fp8(64*W); 1/64 folded downstream

REPLICA_GROUPS = [[0, 1, 2, 3], [4, 5, 6, 7]]


def chunk_meta(c):
    """k-tiles covering the causal+window band for query chunk c, ordered so
    every masked tile sits at the tail, plus the combined-mask slice.

    Returns (tiles, tail, moff, mw): the trailing `tail` tiles are multiplied
    by mkc[:, moff:moff+mw] in one DVE op.  Tail orders: c>=5 -> [c-4 (j<p),
    c (j>=p), sink NLT (p==0)]; c==4 -> [0 ((j<p)|(p==0)), 4 (j>=p)];
    c<=3 -> [c (j>=p)].  (j = query offset in chunk, p = key offset.)
    """
    mid = list(range(max(0, c - 3), c))
    if c >= 5:
        return mid + [c - 4, c, NLT], 3, 0, 384
    if c == 4:
        return mid + [0, 4], 2, 384, 256
    return mid + [c], 1, 640, 128


def _build(with_bias: bool):
    nc = bacc.Bacc("TRN2", target_bir_lowering=False, debug=False, num_devices=NCORES)

    # hi/lo fp8 pairs; weights are packed [h | l] along the column axis so a
    # row's DMA run is 512B (the <512B run penalty doubles DMA cost)
    xh = nc.dram_tensor("xh", [D, L], F8, kind="ExternalInput")
    xl = nc.dram_tensor("xl", [D, L], F8, kind="ExternalInput")
    wq2 = nc.dram_tensor("wq2", [D, 2 * EG], F8, kind="ExternalInput")
    wk2 = nc.dram_tensor("wk2", [D, 2 * EG], F8, kind="ExternalInput")
    wv2 = nc.dram_tensor("wv2", [D, 2 * EG], F8, kind="ExternalInput")
    woT = nc.dram_tensor("woT", [EG, D], BF16, kind="ExternalInput")
    tabs_d = nc.dram_tensor("tabs", [L, 4, EG], BF16, kind="ExternalInput")
    mskd = nc.dram_tensor("msk", [128, 768], BF16, kind="ExternalInput")
    if with_bias:
        bqr = nc.dram_tensor("bqr", [1, EG], BF16, kind="ExternalInput")
        bkr = nc.dram_tensor("bkr", [1, EG], BF16, kind="ExternalInput")
        bvr = nc.dram_tensor("bvr", [1, EG], BF16, kind="ExternalInput")
    outp = nc.dram_tensor("outp", [L, D], BF16, kind="ExternalOutput")
    ssel = nc.dram_tensor("ssel", [128 * 32], F32, kind="Internal")
    ssag = nc.dram_tensor("ssag", [NG, 128 * 32], F32, kind="Internal")

    # partition-major views for blocked DMA loads
    xhv = xh.ap().rearrange("(t p) l -> p t l", p=128)  # [128, 16, 2048]
    xlv = xl.ap().rearrange("(t p) l -> p t l", p=128)
    wqv = wq2.ap().rearrange("(t p) e -> p t e", p=128)  # [128, 16, 512]
    wkv = wk2.ap().rearrange("(t p) e -> p t e", p=128)
    wvv = wv2.ap().rearrange("(t p) e -> p t e", p=128)
    wov = woT.ap().rearrange("(t p) d -> p t d", p=128)  # [128, 2, 2048]
    tbv = tabs_d.ap().rearrange("(lt p) f e -> p lt f e", p=128)  # [128, 16, 4, 256]

    with tile.TileContext(nc) as tc, ExitStack() as ctx:
        consts = ctx.enter_context(tc.tile_pool(name="consts", bufs=1))
        big = ctx.enter_context(tc.tile_pool(name="big", bufs=1))

        ident = consts.tile([128, 128], BF16, tag="ident", name="ident")
        make_identity(nc, ident)
        eps_t = consts.tile([128, 1], F32, tag="eps", name="eps")
        nc.vector.memset(eps_t, EPS)
        one_sc = consts.tile([128, 1], F32, tag="one_sc", name="one_sc")
        nc.vector.memset(one_sc, 1.0)
        ones4 = consts.tile([128, HG], F32, tag="ones4", name="ones4")
        nc.vector.memset(ones4, 1.0)
        mkc = consts.tile([128, 768], BF16, tag="mkc", name="mkc")
        if with_bias:
            ones_row = consts.tile([1, 128], BF16, tag="ones_row", name="ones_row")
            nc.vector.memset(ones_row, 1.0)
            b_rows = {}
            for nm, dram in (("q", bqr), ("k", bkr), ("v", bvr)):
                b_rows[nm] = consts.tile([1, EG], BF16, tag=f"b_{nm}", name=f"b_{nm}")
                nc.sync.dma_start(out=b_rows[nm], in_=dram.ap())

        # persistent tiles
        rq = [big.tile([128, EG], F32, tag=f"rq{i}", name=f"rq{i}") for i in range(NLT)]
        rk = [big.tile([128, EG], F32, tag=f"rk{i}", name=f"rk{i}") for i in range(NLT)]
        # vA[NLT] is the sink tile: zero except row 0 = copy of key 0's row
        vA = [
            big.tile([128, VW], BF16, tag=f"vA{i}", name=f"vA{i}")
            for i in range(NLT + 1)
        ]
        # combined q/k transposed tile, bf16: index j on the middle axis is
        # (0: k-et0, 1: k-et1, 2: q-et0, 3: q-et1).  k rows have an extra
        # 128-col sink tile at [L, L+128): col L = copy of key 0, rest zero
        # (masked out; zeros keep exp() of the dead region finite).
        qkT = big.tile([128, 4, L + 128], BF16, tag="qkT", name="qkT")
        aT = [big.tile([128, L], BF16, tag=f"aT{i}", name=f"aT{i}") for i in range(NET)]
        nc.vector.memset(vA[NLT].bitcast(F32), 0.0)
        for j in range(NET):
            nc.vector.memset(qkT[:, j, L : L + 128].bitcast(F32), 0.0)
        ss2 = big.tile([128, 2 * NLT], F32, tag="ss2", name="ss2")
        ssg = big.tile([128, NG, 2 * NLT], F32, tag="ssg", name="ssg")
        sst = big.tile([128, 2 * NLT], F32, tag="sst", name="sst")
        rr = big.tile([128, 2 * NLT], F32, tag="rr", name="rr")
        wog = big.tile([128, NET, D], BF16, tag="wog", name="wog")

        # ---- projection-phase pools (released before attention) ----
        pctx = ctx.enter_context(ExitStack())
        wp = pctx.enter_context(tc.tile_pool(name="wp", bufs=1))
        xp = pctx.enter_context(tc.tile_pool(name="xp", bufs=1))
        tabsp = pctx.enter_context(tc.tile_pool(name="tabsp", bufs=1))
        work = pctx.enter_context(tc.tile_pool(name="work", bufs=1))
        psP = pctx.enter_context(tc.tile_pool(name="psP", bufs=1, space="PSUM"))

        # x stays resident in SBUF for both phases (64KB/partition fp8 hi+lo);
        # loaded in 512-l-col chunks so every DMA run is >=512B.
        xgh = xp.tile([128, ND, L], F8, tag="xgh", name="xgh")
        xgl = xp.tile([128, ND, L], F8, tag="xgl", name="xgl")

        def load_xchunk(ci):
            cs = slice(ci * 512, (ci + 1) * 512)
            nc.sync.dma_start(out=xgh[:, :, cs], in_=xhv[:, :, cs])
            nc.sync.dma_start(out=xgl[:, :, cs], in_=xlv[:, :, cs])

        def load_tb(lt):
            tb = tabsp.tile([128, 4, EG], BF16, tag="tb", bufs=6, name="tb")
            nc.sync.dma_start(out=tb, in_=tbv[:, lt])
            return tb

        # DMA issue order is SP-queue order: q weights + first x tiles first
        # (quartered, so the first contraction matmuls can start early),
        # the rest behind them.
        wq_g = wp.tile([128, ND, 2 * EG], F8, tag="wq", name="wq")
        wk_g = wp.tile([128, ND, 2 * EG], F8, tag="wk", name="wk")
        wv_g = wp.tile([128, ND, 2 * EG], F8, tag="wv", name="wv")
        nc.sync.dma_start(out=wq_g[:, 0:4], in_=wqv[:, 0:4])
        nc.sync.dma_start(out=xgh[:, 0:4, 0:512], in_=xhv[:, 0:4, 0:512])
        nc.sync.dma_start(out=xgl[:, 0:4, 0:512], in_=xlv[:, 0:4, 0:512])
        nc.sync.dma_start(out=wq_g[:, 4:16], in_=wqv[:, 4:16])
        nc.sync.dma_start(out=xgh[:, 4:16, 0:512], in_=xhv[:, 4:16, 0:512])
        nc.sync.dma_start(out=xgl[:, 4:16, 0:512], in_=xlv[:, 4:16, 0:512])
        nc.sync.dma_start(out=wk_g, in_=wkv)
        tbs_pre = [load_tb(lt) for lt in range(4)]

        def proj_psum(lt, wg, bias_key):
            # fp8 hi/lo: x ~= xh+xl, 64W ~= wh+wl; psum = 64*(x@W) via
            # xh*wh + xh*wl + xl*wh (xl*wl dropped, ~0.4% of a product).
            # DoubleRow packs 2 contraction d-tiles per matmul at 0.5 cyc/row.
            ps = psP.tile([128, EG], F32, tag="pp", bufs=8, name="pp")
            ssl = slice(lt * 128, (lt + 1) * 128)
            for p8 in range(ND // 2):
                d2 = slice(2 * p8, 2 * p8 + 2)
                nc.tensor.matmul(
                    ps, lhsT=xgh[:, d2, ssl], rhs=wg[:, d2, 0:EG],
                    start=(p8 == 0), stop=False, perf_mode=DR,
                )
                nc.tensor.matmul(
                    ps, lhsT=xgh[:, d2, ssl], rhs=wg[:, d2, EG : 2 * EG],
                    start=False, stop=False, perf_mode=DR,
                )
                nc.tensor.matmul(
                    ps, lhsT=xgl[:, d2, ssl], rhs=wg[:, d2, 0:EG],
                    start=False, stop=(p8 == ND // 2 - 1 and not with_bias),
                    perf_mode=DR,
                )
            if with_bias:
                nc.tensor.matmul(
                    ps, lhsT=ones_row, rhs=b_rows[bias_key], start=False, stop=True
                )
            return ps

        # ---------------- phase A: raw q/k projections + rope + partial SS ----
        # squares + rope read the projection PSUM directly (no SBUF staging
        # copy): ACT does the square-accumulate, DVE the rope multiplies.
        for ci in range(4):
            tbs = tbs_pre if ci == 0 else tbs_next
            if ci < 3:
                load_xchunk(ci + 1)
                tbs_next = [load_tb(4 * (ci + 1) + j) for j in range(4)]
            if ci == 2:  # v weights not needed until phase B
                nc.sync.dma_start(out=wv_g, in_=wvv)
            # Q for both l-tiles of a pair BEFORE either K: at startup the
            # first K weights are still in flight behind wq/x, so this gives
            # the PE some Q work to cover the wk transfer.
            for half in range(2):
                lts = (4 * ci + 2 * half, 4 * ci + 2 * half + 1)
                psqs = [proj_psum(lt, wq_g, "q") for lt in lts]
                psks = [proj_psum(lt, wk_g, "k") for lt in lts]
                for sub in range(2):
                    lt = lts[sub]
                    tb = tbs[2 * half + sub]
                    for ps, dst, ti, ss_col in (
                        (psqs[sub], rq[lt], 0, lt),
                        (psks[sub], rk[lt], 2, NLT + lt),
                    ):
                        sq = work.tile([128, EG], F32, tag="sq", bufs=2, name="sq")
                        nc.scalar.activation(
                            sq, ps, mybir.ActivationFunctionType.Square,
                            accum_out=ss2[:, ss_col : ss_col + 1],
                        )
                        tch = tb[:, ti]
                        tsh = tb[:, ti + 1]
                        # rope split across DVE (strided swap muls) and Pool
                        # (straight mul+add); ACT holds the square-accums
                        tmp = work.tile([128, EG], F32, tag="ropetmp", bufs=2, name="ropetmp")
                        nc.vector.tensor_mul(tmp[:, 0::2], ps[:, 1::2], tsh[:, 0::2])
                        nc.vector.tensor_mul(tmp[:, 1::2], ps[:, 0::2], tsh[:, 1::2])
                        nc.vector.tensor_mul(dst, ps, tch)
                        nc.gpsimd.tensor_add(dst, dst, tmp)

        # ---- rmsnorm sum-of-squares completion across the 4-core group ----
        # staging transfers ride the fast HWDGE (sync) queue; phase B no
        # longer loads x, so there is no head-of-line risk there.  wog/masks
        # are queued between them so they aren't stuck behind the
        # collective-dependent gather-in.
        sselv = ssel.ap().rearrange("(p j) -> p j", p=128)  # [128, 32]
        nc.sync.dma_start(out=sselv, in_=ss2)
        nc.gpsimd.collective_compute(
            kind="AllGather",
            op=mybir.AluOpType.bypass,
            replica_groups=REPLICA_GROUPS,
            ins=[ssel.ap()],
            outs=[ssag.ap()],
        )
        nc.sync.dma_start(out=wog, in_=wov)
        nc.sync.dma_start(out=mkc, in_=mskd.ap())
        nc.sync.dma_start(out=ssg, in_=ssag.ap().rearrange("g (p j) -> p g j", p=128))

        # ---------------- phase B: v projection (overlaps the AllGather) ----
        # psum->vA copies on ACT (idle during this phase; on DVE they would
        # delay the rs chain).  The rs = 1/sqrt(mean(ss)+eps) block is
        # injected near the END of phase B: by then the AllGather result is
        # long since landed, so neither ACT nor DVE blocks on it, and rr is
        # ready just before the attention transposes need it.
        for lt in range(NLT):
            if lt == NLT - 4:
                nc.vector.tensor_add(sst, ssg[:, 0], ssg[:, 1])
                nc.vector.tensor_add(sst, sst, ssg[:, 2])
                nc.vector.tensor_add(sst, sst, ssg[:, 3])
                nc.scalar.activation(
                    rr, sst, mybir.ActivationFunctionType.Sqrt,
                    bias=eps_t, scale=1.0 / (INNER * WSCALE * WSCALE),
                )
                nc.vector.reciprocal(rr, rr)
                # dummy Exp: pulls the exp-table load (1.3us) into phase B's
                # ACT idle window instead of the first attention exp.  Both
                # the sqrt and exp tables serve Square/Copy, so these are the
                # only two switches and neither is on the critical path.
                warm = work.tile([1, 1], F32, tag="warm", bufs=1, name="warm")
                nc.scalar.activation(
                    warm, eps_t[0:1, :], mybir.ActivationFunctionType.Exp
                )
            psv = proj_psum(lt, wv_g, "v")
            vA_r = vA[lt].rearrange("p (h c) -> p h c", c=DH + 1)
            nc.scalar.activation(
                vA_r[:, :, 0:DH],
                psv.rearrange("p (h c) -> p h c", c=DH),
                mybir.ActivationFunctionType.Copy,
                scale=1.0 / WSCALE,
            )
            nc.vector.tensor_scalar_mul(vA_r[:, :, DH], ones4, one_sc)
            if lt == 0:
                nc.gpsimd.tensor_copy(vA[NLT][0:1, :], vA[0][0:1, :])

        # ---- release projection pools; open attention/output pools ----
        # PSUM banks are 2KB/partition and pool buffers are bank-granular, so
        # [128, 256]-shaped psums are packed two-per-bank: psS banks hold two
        # k-tiles' scores, psO banks hold two heads' PV outputs, and psP2
        # banks serve both the out-projection and (quartered) the transposes.
        pctx.close()
        esp = ctx.enter_context(tc.tile_pool(name="esp", bufs=1))
        awork = ctx.enter_context(tc.tile_pool(name="awork", bufs=1))
        outw = ctx.enter_context(tc.tile_pool(name="outw", bufs=1))
        # PSUM budget (8 banks): pss 5 (3 manual 768-col slots) + po 1 +
        # pso 1 + ptq(bf16) 1.  Manual sub-bank slots (pool buffers are
        # bank-granular); slot rotation keeps WAR windows wide.  3 score
        # slots let the PE run scores ~3 heads ahead so exps go
        # back-to-back on ACT - the exp pace IS the chunk pace.
        psA = ctx.enter_context(tc.tile_pool(name="psA", bufs=1, space="PSUM"))
        pss_all = psA.tile([128, 2560], F32, tag="pss", bufs=1, name="pss")
        pso_all = psA.tile([DH + 1, 512], F32, tag="pso", bufs=1, name="pso")
        ptq_all = psA.tile([128, 1024], BF16, tag="ptq", bufs=1, name="ptq")
        po_all = psA.tile([128, 512], F32, tag="po", bufs=1, name="po")

        def st_scale(lt):
            """kn/qn scale by rs (to bf16) on Pool; issued at chunk start so
            the result is ready when the PE transposes run mid-chunk."""
            ns = []
            for src_t, col in ((rk[lt], NLT + lt), (rq[lt], lt)):
                n = awork.tile([128, EG], BF16, tag="qkn", bufs=4, name="qkn")
                nc.gpsimd.tensor_scalar_mul(n, src_t, rr[:, col : col + 1])
                ns.append(n)
            return ns

        def st_transpose(lt, ns):
            """4 PE transposes through one psum half-bank, then a single
            strided copy into the combined qkT tile."""
            ptq = ptq_all[:, (lt % 2) * 512 : (lt % 2 + 1) * 512]
            for qi, n in enumerate(ns):
                for et in range(NET):
                    q4 = slice((2 * qi + et) * 128, (2 * qi + et + 1) * 128)
                    nc.tensor.transpose(
                        ptq[:, q4], n[:, et * 128 : (et + 1) * 128], ident
                    )
            nc.vector.tensor_copy(qkT[:, :, lt * 128 : (lt + 1) * 128], ptq)
            if lt == 0:
                for j in range(NET):
                    nc.gpsimd.tensor_copy(qkT[:, j, L : L + 1], qkT[:, j, 0:1])

        def scale_transpose(lt):
            st_transpose(lt, st_scale(lt))

        def outproj_piece(lt, dc):
            # single po slot: pieces are spread across the chunk as PE
            # filler, so the copy->next-matmul WAR serialization is hidden;
            # copies alternate DVE/ACT (the only PSUM readers)
            for et in range(NET):
                nc.tensor.matmul(
                    po_all,
                    lhsT=aT[et][:, lt * 128 : (lt + 1) * 128],
                    rhs=wog[:, et, dc * 512 : (dc + 1) * 512],
                    start=(et == 0),
                    stop=(et == NET - 1),
                )
            osb = outw.tile([128, 512], BF16, tag="osb", bufs=4, name="osb")
            if dc % 2 == 0:
                nc.vector.tensor_copy(osb, po_all)
            else:
                nc.scalar.copy(osb, po_all)
            nc.sync.dma_start(
                out=outp.ap()[lt * 128 : (lt + 1) * 128, dc * 512 : (dc + 1) * 512],
                in_=osb,
            )

        # ---------------- attention, pipelined by query chunk ----------------
        # scale+transposes for chunk c+1 are emitted during chunk c so the
        # DVE-scale -> PE-transpose -> DVE-copy chain hides behind attention.
        def scores(cx, h):
            c, (tiles, tail, moff, mw), cs, es_tiles, _ = cx
            et, r0 = h // 2, (h % 2) * 64
            # one psum tile + one exp instruction for the whole band (<=6
            # tiles); masked tiles sit at the tail so ONE combined-mask mul
            # covers them all
            n = len(tiles)
            slot = (4 * c + h) % 3
            pss = pss_all[:, slot * 768 : slot * 768 + 768]
            esb = esp.tile([128, 768], BF16, tag="es", bufs=6, name="es")
            for idx, t in enumerate(tiles):
                hs = slice(idx * CH, (idx + 1) * CH)
                nc.tensor.matmul(
                    pss[:, hs],
                    lhsT=qkT[r0 : r0 + 64, et, t * 128 : (t + 1) * 128],
                    rhs=qkT[r0 : r0 + 64, 2 + et, cs],
                    start=True,
                    stop=True,
                    skip_group_check=True,
                )
                es_tiles[(h, t)] = (esb, hs)
            nc.scalar.activation(
                esb[:, 0 : n * CH], pss[:, 0 : n * CH],
                mybir.ActivationFunctionType.Exp,
            )
            s0 = (n - tail) * CH
            nc.vector.tensor_mul(
                esb[:, s0 : s0 + mw], esb[:, s0 : s0 + mw], mkc[:, moff : moff + mw]
            )

        def pv(cx, h):
            c, (tiles, tail, moff, mw), cs, es_tiles, pso_banks = cx
            et, r0 = h // 2, (h % 2) * 64
            off = (h // 2) * 256 + (h % 2) * CH
            pso = pso_all[:, off : off + CH]
            for i, t in enumerate(tiles):
                esb, hs = es_tiles[(h, t)]
                nc.tensor.matmul(
                    pso,
                    lhsT=vA[t][:, h * (DH + 1) : (h + 1) * (DH + 1)],
                    rhs=esb[:, hs],
                    start=(i == 0),
                    stop=(i == len(tiles) - 1),
                    skip_group_check=True,
                )

        def denom(cx, half):
            # reciprocal over 2 heads' ones-column row, one Pool broadcast,
            # then per-head normalize muls on DVE (psum reads are ACT/DVE only)
            c, _, cs, es_tiles, pso_banks = cx
            pso = pso_all[:, half * 256 : half * 256 + 2 * CH]
            rec = awork.tile([1, 2 * CH], F32, tag="rec", bufs=3, name="rec")
            nc.vector.reciprocal(rec, pso[DH : DH + 1, :])
            rb = awork.tile([64, 2 * CH], F32, tag="rb", bufs=3, name="rb")
            nc.gpsimd.partition_broadcast(rb, rec)
            for hh in range(2):
                h = 2 * half + hh
                et, r0 = h // 2, (h % 2) * 64
                nc.vector.tensor_mul(
                    aT[et][r0 : r0 + 64, cs],
                    pso[0:DH, hh * CH : (hh + 1) * CH],
                    rb[:, hh * CH : (hh + 1) * CH],
                )

        def mkcx(c):
            return (c, chunk_meta(c), slice(c * CH, (c + 1) * CH), {}, {})

        # the lagged out-projection of chunk c-1 is interleaved into chunk c's
        # score/PV sequence: its matmuls depend only on chunk c-1's aT, so
        # they are ideal PE filler while PV waits on the exp/mask chain.
        scale_transpose(0)
        scale_transpose(1)
        for c in range(NCH):
            cx = mkcx(c)
            # PE queue is in-order: interleave so nothing head-of-line blocks.
            # scores(h+3) waits on exp(h) freeing its pss slot; outproj
            # pieces (lagged 2 chunks so the denom chain is long done) are
            # filler between score/pv groups.
            ns = st_scale(c + 2) if c + 2 < NCH else None
            scores(cx, 0)
            scores(cx, 1)
            if c >= 2:
                outproj_piece(c - 2, 0)
            scores(cx, 2)
            pv(cx, 0)
            if ns is not None:
                st_transpose(c + 2, ns)
            if c >= 2:
                outproj_piece(c - 2, 1)
            scores(cx, 3)
            pv(cx, 1)
            denom(cx, 0)
            if c >= 2:
                outproj_piece(c - 2, 2)
            pv(cx, 2)
            pv(cx, 3)
            denom(cx, 1)
            if c >= 2:
                outproj_piece(c - 2, 3)
        for lt in (NCH - 2, NCH - 1):
            for dc in range(NDC):
                outproj_piece(lt, dc)

    nc.compile()
    return nc


_NC_CACHE = {}


def get_nc(with_bias: bool):
    if with_bias not in _NC_CACHE:
        _NC_CACHE[with_bias] = _build(with_bias)
    return _NC_CACHE[with_bias]


def _fold_tables(cosf, sinf, w):
    """Fold rmsnorm weight w (per channel) into interleaved-rope cos/sin tables.

    Kernel computes raw*tc + swap(raw)*ts with swap pairing (odd->even,
    even->odd), so:
      tc[:, e]    = cos[:, e]    * w[e]
      ts[:, 2i]   = -sin[:, 2i]  * w[2i+1]
      ts[:, 2i+1] =  sin[:, 2i+1]* w[2i]
    """
    tc_ = cosf * w[None, :]
    ts_ = np.empty_like(sinf)
    ts_[:, 0::2] = -sinf[:, 0::2] * w[None, 1::2]
    ts_[:, 1::2] = sinf[:, 1::2] * w[None, 0::2]
    return tc_, ts_


def _build_masks():
    """Concatenated mask strip [128, 768]: [j<p | j>=p | p==0] for c>=5 at 0,
    [(j<p)|(p==0) | j>=p] for c==4 at 384, [j>=p] for c<=3 at 640."""
    p = np.arange(128)[:, None]
    j = np.arange(CH)[None, :]
    lower = (j < p).astype(np.float32)
    diag = (j >= p).astype(np.float32)
    sink = np.broadcast_to((p == 0).astype(np.float32), (128, CH))
    lower_sink = ((j < p) | (p == 0)).astype(np.float32)
    strip = np.concatenate([lower, diag, sink, lower_sink, diag, diag], axis=1)
    return np.ascontiguousarray(strip).astype(ml_dtypes.bfloat16)


def make_in_maps(inputs):
    f = lambda k: np.asarray(inputs[k], np.float32)
    x = f("x")
    cosf = f("cos")[0]
    sinf = f("sin")[0]
    lls = f("logit_log_scale")[0, :, 0]
    bq, bk, bv = f("bq"), f("bk"), f("bv")
    with_bias = bool(np.any(bq) or np.any(bk) or np.any(bv))

    qtc_f, qts_f = _fold_tables(cosf, sinf, f("qn_w"))
    ktc_f, kts_f = _fold_tables(cosf, sinf, f("kn_w"))
    qs = (lls * (1.0 / np.sqrt(DH)))[:, None].astype(np.float32)
    # tables absorb the 1/WSCALE that undoes the fp8 weight scaling
    tabs_full = np.stack(
        [qtc_f * qs, qts_f * qs, ktc_f, kts_f], axis=1
    ) * (1.0 / WSCALE)  # [L, 4, INNER]

    Wq, Wk, Wv, Wo = f("Wq"), f("Wk"), f("Wv"), f("Wo")
    msk = _build_masks()

    FP8 = ml_dtypes.float8_e4m3

    def hilo(a):
        h = a.astype(FP8)
        l = (a - h.astype(np.float32)).astype(FP8)
        return h, l

    x_b = []
    for b in range(B):
        xh_, xl_ = hilo(np.ascontiguousarray(x[b].T))
        x_b.append({"xh": xh_, "xl": xl_})
    per_g = []
    for g in range(NG):
        sl = slice(g * EG, (g + 1) * EG)
        gm = {"msk": msk}
        for nm, W in (("wq", Wq), ("wk", Wk), ("wv", Wv)):
            h, l = hilo(np.ascontiguousarray(W[sl].T) * WSCALE)
            gm[nm + "2"] = np.ascontiguousarray(np.concatenate([h, l], axis=1))
        gm["woT"] = np.ascontiguousarray(Wo[:, sl].T).astype(ml_dtypes.bfloat16)
        gm["tabs"] = np.ascontiguousarray(tabs_full[:, :, sl]).astype(ml_dtypes.bfloat16)
        if with_bias:
            gm["bqr"] = (WSCALE * bq)[None, sl].astype(ml_dtypes.bfloat16)
            gm["bkr"] = (WSCALE * bk)[None, sl].astype(ml_dtypes.bfloat16)
            gm["bvr"] = (WSCALE * bv)[None, sl].astype(ml_dtypes.bfloat16)
        per_g.append(gm)

    ims = []
    for c in range(NCORES):
        b, g = divmod(c, NG)
        ims.append({**x_b[b], **per_g[g]})
    return ims, with_bias


last_results = None


def kernel(**inputs):
    global last_results
    ims, with_bias = make_in_maps(inputs)
    nc = get_nc(with_bias)
    res = run_bass_kernel_spmd(nc, ims, core_ids=list(range(NCORES)))
    last_results = res
    out = np.zeros((B, L, D), np.float32)
    for c, om in enumerate(res.results):
        out[c // NG] += np.asarray(om["outp"], dtype=np.float32)
    out += np.asarray(inputs["bo"], np.float32)[None, None, :]
    return out

